# revision 21
# baseline (speedup 1.0000x reference)
"""ContraCLM token-level contrastive loss on 8 Trainium2 NeuronCores.

Data-parallel over the batch: core b handles sample b (B=8). Host-side,
each sample's unmasked tokens are compacted to the front (a pure gather /
layout transform; the kernel still sees real data rows for pads so norms
never hit 0/0) and padded to P=1024 (n ~ Binomial(1536, .5) ~ 768; the
build is generic in P with a P=1536 fallback if some n > 1024).

Per core, with P=1024, D=1024, T=0.05:

  f_v = l2norm(h_v) with pad rows zeroed (mask folded into the reciprocal
  norm scale); G_v = (8*f_v)^T stored [D, P] in fp8e4 (x8 keeps entries
  in e4m3's normal range).

  The 2P x 2P similarity matrix [[A B];[B^T C]] (A = f1 f1^T etc.) is
  symmetric, so only A/C upper-triangle strips and all of B are computed
  as [128, 512] PSUM strips (fp8 DoubleRow, K=1024). exp(sim/T) row sums
  come from the ScalarE activation free-dim accumulator; strips containing
  the diagonal get a strict-upper affine_select then a DVE row-sum. The
  mirrored (lower-triangle) contributions are recovered from column sums:
  a ones-vector stationary matmul streams each es strip into a per-column
  [1, 512] PSUM accumulator, which is transposed to token-major layout at
  the end via K=1 outer-product matmuls.

  B's diagonal is exp(pos_sim/T): it is left inside the row/col sums
  (denominator = Ng + pos exactly), and 20*pos_sim for the numerator is
  extracted exactly from the f32 PSUM sim diagonal with a fused
  tensor_tensor_reduce against an identity tile.

  Pad columns contribute exp(0)=1 to every row sum: subtract
  K0 = 2P - 2n. per_tok = ln(denom) - 20*pos_sim, masked mean over 2n
  tokens; per-sample means are averaged on the host (no collective).
"""

import sys

for _p in ("/opt/trn_rl_repo", "/opt/pypackages"):
    if _p not in sys.path:
        sys.path.append(_p)

from contextlib import ExitStack

import numpy as np

import bass_rust

import concourse.bass as bass
import concourse.tile as tile
from concourse import mybir
from concourse.bass_utils import run_bass_kernel_spmd
from concourse.masks import make_identity
from concourse.vector_clock import ScopedClock

# The walrus build in this container encodes at most 2 sync waits per
# instruction (bass_rust's inst_waits_full agrees), but Tile's semaphore
# assignment can attach more. Hoist excess waits onto unfusable same-engine
# NoOps immediately before the instruction — the engine executes its queue
# in order, so semantics are preserved.
_MAX_WAITS = 1


def _split_excess_waits(nc, ordered):
    for bb_name, insts in ordered.items():
        out = []
        changed = False
        for inst in insts:
            si = getattr(inst, "sync_info", None)
            waits = list(si.on_wait) if si is not None else []
            if len(waits) > _MAX_WAITS:
                changed = True
                extra, keep = waits[:-_MAX_WAITS], waits[-_MAX_WAITS:]
                for i in range(0, len(extra), _MAX_WAITS):
                    out.append(mybir.InstNoOp(
                        name=nc.get_next_instruction_name(),
                        sync_info=mybir.SyncInfo(
                            on_wait=extra[i:i + _MAX_WAITS], on_update=[]),
                        bass_nofuse=True,
                        engine=inst.engine,
                    ))
                si.on_wait = keep
            out.append(inst)
        if changed:
            insts[:] = out


_orig_lower_ordered_insts = tile.TileContext._lower_ordered_insts


def _patched_lower_ordered_insts(self, ordered):
    _split_excess_waits(self.nc, ordered)
    return _orig_lower_ordered_insts(self, ordered)


tile.TileContext._lower_ordered_insts = _patched_lower_ordered_insts


def _split_waits_drain_and_barrier(self, tick_clock, wait_clock):
    nc = self.nc
    probe = nc.sync.nop(nofuse=True)
    wait_clock.add_sem_waits(
        probe.ins, ScopedClock({None: tick_clock.global_clock}))
    si = probe.ins.sync_info
    waits = list(si.on_wait) if si is not None else []
    if len(waits) > _MAX_WAITS:
        si.on_wait = waits[:_MAX_WAITS]
        for i in range(_MAX_WAITS, len(waits), _MAX_WAITS):
            nxt = nc.sync.nop(nofuse=True)
            nxt.ins.sync_info = bass_rust.SyncInfo(
                on_wait=waits[i:i + _MAX_WAITS], on_update=[])
    nc.sync.drain()
    nc.all_engine_barrier()
    assert self.sems is not None
    popped = nc._tile_sem_poison_stack.pop()
    assert popped is self._sem_poison
    nc.clear_and_free_semaphores(list(self.sems.allocated().values()))
    nc.all_engine_barrier()


tile.TileContext._drain_and_barrier = _split_waits_drain_and_barrier

S, D, NCORES = 1536, 1024, 8
P_MAIN = 1024            # compacted+padded tokens per view
KT = D // 128            # 8 contraction tiles
TEMP_INV = 20.0          # 1 / 0.05
FP8_SCALE = 8.0          # f entries ~N(0, 1/32); x8 keeps them in e4m3's
                         # normal range (|f|*8 <~ 2, well under 240)
F32 = mybir.dt.float32
BF16 = mybir.dt.bfloat16
FP8 = mybir.dt.float8e4
AF = mybir.ActivationFunctionType
ALU = mybir.AluOpType
DR = mybir.MatmulPerfMode.DoubleRow


def _build(p: int, num_devices: int = NCORES) -> bass.Bass:
    PT = p // 128        # token tiles per view
    NB2 = 2 * PT         # block rows of the 2P x 2P matrix
    half = p // 512      # column strips per view
    exp_scale = TEMP_INV / (FP8_SCALE * FP8_SCALE)
    pos_scale = TEMP_INV / (FP8_SCALE * FP8_SCALE)

    nc = bass.Bass(num_devices=num_devices)
    h1 = nc.dram_tensor("h1", [p, D], F32, kind="ExternalInput")
    h2 = nc.dram_tensor("h2", [p, D], F32, kind="ExternalInput")
    # mask, pre-laid-out host-side as [128, PT] so token t = 128*col + row
    maskT = nc.dram_tensor("maskT", [128, PT], F32, kind="ExternalInput")
    out = nc.dram_tensor("loss", [1, 1], F32, kind="ExternalOutput")

    with tile.TileContext(nc) as tc, ExitStack() as ctx:
        const_pool = ctx.enter_context(tc.tile_pool(name="const", bufs=1))
        big = ctx.enter_context(tc.tile_pool(name="big", bufs=1))
        stat = ctx.enter_context(tc.tile_pool(name="stat", bufs=1))

        identB = const_pool.tile([128, 128], BF16)
        make_identity(nc, identB[:])
        ones_col = const_pool.tile([128, 1], F32)
        nc.gpsimd.memset(ones_col[:], 1.0)
        ones_sq = const_pool.tile([128, 128], F32)
        nc.gpsimd.memset(ones_sq[:], 1.0)
        ones_bf = const_pool.tile([128, 1], BF16)
        nc.gpsimd.memset(ones_bf[:], 1.0)
        one_f32 = const_pool.tile([1, 1], F32)
        nc.gpsimd.memset(one_f32[:], 1.0)
        msk = const_pool.tile([128, PT], F32)
        nc.sync.dma_start(msk[:], maskT[:])

        # tile-major transposed features: fT[:, t, k*128+c] holds
        # (8*f)^T[d = k*128 + partition, token = t*128 + c] in fp8e4, so
        # each token tile's transpose lands as one contiguous copy
        fT1 = big.tile([128, PT, KT * 128], FP8)
        fT2 = big.tile([128, PT, KT * 128], FP8)
        acc = stat.tile([128, NB2, 2 * half], F32)   # per-strip row sums
        poss20 = stat.tile([128, PT], F32)       # 20 * pos_sim
        csum_sb = stat.tile([1, 2 * p], F32)     # mirror column sums
        msk24 = stat.tile([128, NB2], F32)
        negK0 = stat.tile([128, 1], F32)
        recn = stat.tile([1, 1], F32)

        # zero row-sum slots never written (below-diagonal A/C strips)
        nc.gpsimd.memset(acc[:], 0.0)

        # ---- HAM warmup: keep the PE busy from t~0 so it upclocks.
        # The result is read and spilled to a DRAM scratch so dead-code
        # elimination cannot drop the matmuls.
        wdump = nc.dram_tensor("wdump", [1, 1], F32, kind="Internal")
        with tc.tile_pool(name="warm", bufs=1, space="PSUM") as wp:
            wps = wp.tile([128, 128], F32)
            for i in range(24):
                nc.tensor.matmul(wps[:], identB[:], identB[:],
                                 start=(i == 0), stop=(i == 23))
            wdbg = stat.tile([1, 1], F32)
            nc.vector.tensor_copy(wdbg[:], wps[0:1, 0:1])
            nc.sync.dma_start(wdump[:], wdbg[:])

        # ---- phase 0: mask-only precomputes ----
        with tc.tile_pool(name="ep0", bufs=1) as ep0, \
             tc.tile_pool(name="ep0_ps", bufs=1, space="PSUM") as ep0p:
            msum = ep0.tile([128, 1], F32)
            nc.vector.tensor_reduce(msum[:], msk[:],
                                    axis=mybir.AxisListType.X, op=ALU.add)
            nps = ep0p.tile([128, 1], F32)
            nc.tensor.matmul(nps[:], ones_sq[:], msum[:], start=True,
                             stop=True)
            # -K0 = 2n - 2P, broadcast to all partitions
            nc.scalar.activation(negK0[:], nps[:], AF.Copy, scale=2.0,
                                 bias=float(-2 * p))
            n2c = ep0.tile([1, 1], F32)
            nc.scalar.activation(n2c[:], nps[0:1, :], AF.Copy, scale=2.0)
            nc.vector.reciprocal(recn[:], n2c[:])   # 1/(2n)
            nc.vector.tensor_copy(msk24[:, 0:PT], msk[:])
            nc.vector.tensor_copy(msk24[:, PT:NB2], msk[:])

        with tc.tile_pool(name="mm_ps", bufs=2, space="PSUM") as mmp, \
             tc.tile_pool(name="cng_ps", bufs=1, space="PSUM") as cngp, \
             tc.tile_pool(name="es", bufs=4) as esp, \
             tc.tile_pool(name="ht", bufs=3) as htp, \
             tc.tile_pool(name="scr", bufs=2) as scr, \
             tc.tile_pool(name="sc", bufs=4) as scp, \
             tc.tile_pool(name="tt", bufs=2) as ttp:

            cng = cngp.tile([128, NB2], F32)     # mirror sums, token-major
            pending = []                         # deferred colsum matmuls

            def flush_pending():
                while pending:
                    pending.pop(0)()

            def emit_tile(v, t, src_dram, fT, tps):
                ht = htp.tile([128, D], F32, tag="ht", name=f"ht{v}_{t}")
                nc.sync.dma_start(ht[:], src_dram[t * 128:(t + 1) * 128, :])
                sq = scr.tile([128, D], BF16, tag="sq", name=f"sq{v}_{t}")
                ss = scp.tile([128, 1], F32, tag="ss", name=f"ss{v}_{t}")
                nc.vector.scalar_tensor_tensor(
                    out=sq[:], in0=ht[:], scalar=1.0, in1=ht[:],
                    op0=ALU.mult, op1=ALU.mult, accum_out=ss[:])
                # 1/sqrt(ss) = exp(-0.5*ln(ss)): Ln/Exp share a ScalarE
                # table with the strip exps, so no ACT_TABLE_LOAD thrash
                lnss = scp.tile([128, 1], F32, tag="ln", name=f"ln{v}_{t}")
                nc.scalar.activation(lnss[:], ss[:], AF.Ln)
                sc = scp.tile([128, 1], F32, tag="sc", name=f"sc{v}_{t}")
                nc.scalar.activation(sc[:], lnss[:], AF.Exp, scale=-0.5)
                scm = scp.tile([128, 1], F32, tag="scm", name=f"scm{v}_{t}")
                nc.vector.tensor_mul(scm[:], sc[:], msk[:, t:t + 1])
                fnb = scr.tile([128, D], BF16, tag="fn", name=f"fn{v}_{t}")
                nc.vector.tensor_scalar_mul(fnb[:], ht[:], scm[:])
                pt = tps.tile([128, D], BF16, tag="pt", name=f"pt{v}_{t}")
                for k in range(KT):
                    nc.tensor.transpose(pt[:, k * 128:(k + 1) * 128],
                                        fnb[:, k * 128:(k + 1) * 128],
                                        identB[:])
                # quantize to fp8 (x8) while moving PSUM->SBUF; the
                # tile-major fT makes this a contiguous copy
                nc.scalar.activation(fT[:, t, :], pt[:], AF.Copy,
                                     scale=FP8_SCALE)

            def emit_strip(r, csT, quad, cs_ps, first, last):
                """One sim strip at block-row r (global), local column
                strip csT of quadrant quad. A/C strips containing the
                diagonal are narrowed to skip fully-below-diagonal blocks."""
                lhsT = fT1 if r < PT else fT2
                rT = r % PT
                rhsT = fT1 if quad == "A" else fT2
                ko = 0                            # leading blocks skipped
                if quad != "B" and csT * 512 - rT * 128 < 128:
                    ko = rT - 4 * csT
                nw = 512 - 128 * ko               # strip width
                ps = mmp.tile([128, 512], F32, tag="ps",
                              name=f"ps{quad}_{csT}_{r}")
                rhs4 = rhsT[:, 4 * csT + ko:4 * csT + 4, :].rearrange(
                    "q t (k c) -> q k t c", k=KT)
                lhs3 = lhsT[:, rT, :].rearrange("q (k c) -> q k c", k=KT)
                for g in range(KT // 2):
                    nc.tensor.matmul(
                        ps[:, 0:nw],
                        lhs3[:, 2 * g:2 * g + 2, :],
                        rhs4[:, 2 * g:2 * g + 2, :, :],
                        perf_mode=DR,
                        start=(g == 0), stop=(g == KT // 2 - 1))
                # previous strip's colsum lands here: by now its exp result
                # is ready, so the PE doesn't stall on it
                flush_pending()
                es = esp.tile([128, 512], BF16, tag="es",
                              name=f"es{quad}_{csT}_{r}")
                cs_g = csT if quad == "A" else half + csT
                if quad == "B":
                    nc.scalar.activation(es[:], ps[:], AF.Exp,
                                         scale=exp_scale,
                                         accum_out=acc[:, r, cs_g:cs_g + 1])
                    if csT * 4 <= rT <= csT * 4 + 3:
                        # pos_sim lives on this strip's diagonal block:
                        # extract it exactly from the f32 PSUM sims
                        jb = rT - csT * 4
                        sct = ttp.tile([128, 128], F32, tag="sct",
                                       name=f"sct_{r}")
                        nc.vector.scalar_tensor_tensor(
                            out=sct[:],
                            in0=ps[:, jb * 128:(jb + 1) * 128],
                            scalar=pos_scale,
                            in1=identB[:],
                            op0=ALU.mult,
                            op1=ALU.mult,
                            accum_out=poss20[:, rT:rT + 1])
                elif ko == 0 and csT * 512 - rT * 128 >= 128:
                    # strictly above the diagonal: plain exp + row sums
                    nc.scalar.activation(
                        es[:], ps[:], AF.Exp, scale=exp_scale,
                        accum_out=acc[:, r, cs_g:cs_g + 1])
                else:
                    # first block is the diagonal one: strict upper keep
                    nc.scalar.activation(es[:, 0:nw], ps[:, 0:nw], AF.Exp,
                                         scale=exp_scale)
                    # keep col > row: -1 + (-1)*p + 1*c >= 0
                    nc.gpsimd.affine_select(
                        out=es[:, 0:nw], in_=es[:, 0:nw],
                        compare_op=ALU.is_ge,
                        fill=0.0, base=-1, pattern=[[1, nw]],
                        channel_multiplier=-1)
                    nc.vector.tensor_reduce(acc[:, r, cs_g:cs_g + 1],
                                            es[:, 0:nw],
                                            axis=mybir.AxisListType.X,
                                            op=ALU.add)

                def colsum(es=es, first=first, last=last, vec=cs_ps,
                           ko=ko, nw=nw):
                    nc.tensor.matmul(vec[0:1, ko * 128:ko * 128 + nw],
                                     ones_bf[:], es[:, 0:nw],
                                     start=first, stop=last,
                                     skip_group_check=True)
                pending.append(colsum)

            def emit_mirror(base_chunk, nchunks):
                # transpose csum_sb chunks to token-major via K=1 matmuls
                for c in range(base_chunk, base_chunk + nchunks):
                    nc.tensor.matmul(cng[:, c:c + 1],
                                     csum_sb[0:1, c * 128:(c + 1) * 128],
                                     one_f32[:], start=True, stop=True)

            def a_strips(csT):
                return [(r, "A") for r in range(min(4 * csT + 4, PT))]

            def bc_strips(csT):
                # C diag strips first (their exp->affine->reduce chain
                # overlaps later strips; the emission leader is full
                # width, as the PSUM colsum accumulation group needs),
                # pure-upper C next, B strips last so the drain tail
                # only needs cheap exp+accum
                crows = list(range(min(4 * csT + 4, PT)))
                diag = [(PT + rc, "C") for rc in crows
                        if csT * 512 - rc * 128 < 128]
                pure = [(PT + rc, "C") for rc in crows
                        if csT * 512 - rc * 128 >= 128]
                return diag + pure + [(r, "B") for r in range(PT)]

            # ---- merged schedule: all 16 tiles in sequence, ready
            # strips spread across the remaining tile slots so the PE
            # never starves while tiles DMA/normalize/transpose ----
            with tc.tile_pool(name="tp_ps", bufs=1, space="PSUM") as tps, \
                 tc.tile_pool(name="psA", bufs=1, space="PSUM") as psAp, \
                 tc.tile_pool(name="psB", bufs=1, space="PSUM") as psBp:
                psA = [psAp.tile([1, 512], F32, name=f"psA{c}")
                       for c in range(half)]
                psB = [psBp.tile([1, 512], F32, name=f"psB{c}")
                       for c in range(half)]
                # group bookkeeping: when the last colsum of a column
                # group has been emitted, immediately fold its [1,512]
                # accumulator to token-major mirror columns
                remaining = {}
                mirror_pending = []   # [countdown, closure]

                def tick_mirrors():
                    for ent in list(mirror_pending):
                        ent[0] -= 1
                        if ent[0] <= 0:
                            mirror_pending.remove(ent)
                            ent[1]()

                def finish_group(v, csT):
                    flush_pending()
                    vec = (psA if v == 1 else psB)[csT]
                    base = 0 if v == 1 else p
                    nc.vector.tensor_copy(
                        csum_sb[0:1, base + csT * 512:base + (csT + 1) * 512],
                        vec[:])
                    chunk0 = (0 if v == 1 else PT) + 4 * csT
                    # the K=1 fold matmuls wait on the DVE copy above;
                    # defer them a couple of strips so the PE never stalls
                    mirror_pending.append(
                        [2, lambda c=chunk0: emit_mirror(c, 4)])

                def do_strip(s):
                    v, csT = (1 if s[2] == "A" else 2), s[1]
                    emit_strip(s[0], s[1], s[2], s[3], s[4], s[5])
                    tick_mirrors()
                    remaining[(v, csT)] -= 1
                    if remaining[(v, csT)] == 0:
                        finish_group(v, csT)

                queue = []
                tiles = [(1, t, h1, fT1) for t in range(PT)] + \
                        [(2, t, h2, fT2) for t in range(PT)]
                for i, (v, t, srcd, fT) in enumerate(tiles):
                    emit_tile(v, t, srcd, fT, tps)
                    flush_pending()
                    if t % 4 == 3:
                        csT = t // 4
                        group = a_strips(csT) if v == 1 else bc_strips(csT)
                        vecs = psA if v == 1 else psB
                        remaining[(v, csT)] = len(group)
                        for j, (r, qd) in enumerate(group):
                            queue.append((r, csT, qd, vecs[csT],
                                          j == 0, j == len(group) - 1))
                    tiles_left = len(tiles) - 1 - i
                    if tiles_left > 0 and queue:
                        n_emit = -(-len(queue) // (tiles_left + 1))
                        n_emit = min(len(queue), max(n_emit, 2))
                        for _ in range(n_emit):
                            do_strip(queue.pop(0))
                for s in queue:
                    do_strip(s)
                flush_pending()
                for ent in mirror_pending:
                    ent[1]()
                mirror_pending.clear()

            # ---- epilogue: final reduction chain ----
            with tc.tile_pool(name="ep", bufs=1) as ep, \
                 tc.tile_pool(name="ep_ps", bufs=1, space="PSUM") as epp:
                cngs = ep.tile([128, NB2], F32)
                nc.vector.tensor_copy(cngs[:], cng[:])
                ng = ep.tile([128, NB2], F32)
                nc.vector.tensor_reduce(ng[:], acc[:],
                                        axis=mybir.AxisListType.X,
                                        op=ALU.add)
                den = ep.tile([128, NB2], F32)
                nc.vector.tensor_add(den[:], ng[:], cngs[:])
                nc.vector.tensor_scalar_add(den[:], den[:], negK0[:])
                lg = ep.tile([128, NB2], F32)
                nc.scalar.activation(lg[:], den[:], AF.Ln)
                pm = ep.tile([128, NB2], F32)
                nc.vector.tensor_copy(pm[:, 0:PT], poss20[:])
                nc.vector.tensor_copy(pm[:, PT:NB2], poss20[:])
                d1 = ep.tile([128, NB2], F32)
                nc.vector.tensor_sub(d1[:], lg[:], pm[:])
                ptok = ep.tile([128, NB2], F32)
                tsum = ep.tile([128, 1], F32)
                nc.vector.scalar_tensor_tensor(
                    out=ptok[:], in0=d1[:], scalar=1.0, in1=msk24[:],
                    op0=ALU.mult, op1=ALU.mult, accum_out=tsum[:])
                lps = epp.tile([1, 1], F32)
                nc.tensor.matmul(lps[:], ones_col[:], tsum[:], start=True,
                                 stop=True)
                lsb = ep.tile([1, 1], F32)
                nc.vector.tensor_mul(lsb[:], lps[:], recn[:])
                nc.sync.dma_start(out[:], lsb[:])

    return nc


_NC = {}


def _get_nc(p: int) -> bass.Bass:
    if p not in _NC:
        _NC[p] = _build(p)
    return _NC[p]


def _mask_layout(mask_col: np.ndarray, p: int) -> np.ndarray:
    # token t = 128 * col + row  ->  [128, PT]
    return np.ascontiguousarray(
        mask_col.astype(np.float32).reshape(p // 128, 128).T)


def _in_maps(h1, h2, mask, p):
    maps = []
    for b in range(NCORES):
        idx = np.argsort(~mask[b], kind="stable")[:p]
        maps.append({
            "h1": np.ascontiguousarray(h1[b][idx]),
            "h2": np.ascontiguousarray(h2[b][idx]),
            "maskT": _mask_layout(mask[b][idx], p),
        })
    return maps


def kernel(last_hidden_states_1, last_hidden_states_2, token_mask_batch):
    h1 = np.ascontiguousarray(np.asarray(last_hidden_states_1,
                                         dtype=np.float32))
    h2 = np.ascontiguousarray(np.asarray(last_hidden_states_2,
                                         dtype=np.float32))
    mask = np.asarray(token_mask_batch).astype(bool)
    assert h1.shape == (NCORES, S, D), h1.shape

    p = P_MAIN if int(mask.sum(axis=1).max()) <= P_MAIN else S
    nc = _get_nc(p)
    res = run_bass_kernel_spmd(nc, _in_maps(h1, h2, mask, p),
                               list(range(NCORES)))
    vals = [np.asarray(res.results[b]["loss"], dtype=np.float32).reshape(())
            for b in range(NCORES)]
    return np.float32(np.mean(vals))


# revision 22
# speedup vs baseline: 1.0331x; 1.0331x over previous
"""ContraCLM token-level contrastive loss on 8 Trainium2 NeuronCores.

Data-parallel over the batch: core b handles sample b (B=8). Host-side,
each sample's unmasked tokens are compacted to the front (a pure gather /
layout transform; the kernel still sees real data rows for pads so norms
never hit 0/0) and padded to P=1024 (n ~ Binomial(1536, .5) ~ 768; the
build is generic in P with a P=1536 fallback if some n > 1024).

Per core, with P=1024, D=1024, T=0.05:

  f_v = l2norm(h_v) with pad rows zeroed (mask folded into the reciprocal
  norm scale); G_v = (8*f_v)^T stored [D, P] in fp8e4 (x8 keeps entries
  in e4m3's normal range).

  The 2P x 2P similarity matrix [[A B];[B^T C]] (A = f1 f1^T etc.) is
  symmetric, so only A/C upper-triangle strips and all of B are computed
  as [128, 512] PSUM strips (fp8 DoubleRow, K=1024). exp(sim/T) row sums
  come from the ScalarE activation free-dim accumulator; strips containing
  the diagonal get a strict-upper affine_select then a DVE row-sum. The
  mirrored (lower-triangle) contributions are recovered from column sums:
  a ones-vector stationary matmul streams each es strip into a per-column
  [1, 512] PSUM accumulator, which is transposed to token-major layout at
  the end via K=1 outer-product matmuls.

  B's diagonal is exp(pos_sim/T): it is left inside the row/col sums
  (denominator = Ng + pos exactly), and 20*pos_sim for the numerator is
  extracted exactly from the f32 PSUM sim diagonal with a fused
  tensor_tensor_reduce against an identity tile.

  Pad columns contribute exp(0)=1 to every row sum: subtract
  K0 = 2P - 2n. per_tok = ln(denom) - 20*pos_sim, masked mean over 2n
  tokens; per-sample means are averaged on the host (no collective).
"""

import sys

for _p in ("/opt/trn_rl_repo", "/opt/pypackages"):
    if _p not in sys.path:
        sys.path.append(_p)

from contextlib import ExitStack

import numpy as np

import bass_rust

import concourse.bass as bass
import concourse.tile as tile
from concourse import mybir
from concourse.bass_utils import run_bass_kernel_spmd
from concourse.masks import make_identity
from concourse.vector_clock import ScopedClock

# The walrus build in this container encodes at most 2 sync waits per
# instruction (bass_rust's inst_waits_full agrees), but Tile's semaphore
# assignment can attach more. Hoist excess waits onto unfusable same-engine
# NoOps immediately before the instruction — the engine executes its queue
# in order, so semantics are preserved.
_MAX_WAITS = 1


def _split_excess_waits(nc, ordered):
    for bb_name, insts in ordered.items():
        out = []
        changed = False
        for inst in insts:
            si = getattr(inst, "sync_info", None)
            waits = list(si.on_wait) if si is not None else []
            if len(waits) > _MAX_WAITS:
                changed = True
                extra, keep = waits[:-_MAX_WAITS], waits[-_MAX_WAITS:]
                for i in range(0, len(extra), _MAX_WAITS):
                    out.append(mybir.InstNoOp(
                        name=nc.get_next_instruction_name(),
                        sync_info=mybir.SyncInfo(
                            on_wait=extra[i:i + _MAX_WAITS], on_update=[]),
                        bass_nofuse=True,
                        engine=inst.engine,
                    ))
                si.on_wait = keep
            out.append(inst)
        if changed:
            insts[:] = out


_orig_lower_ordered_insts = tile.TileContext._lower_ordered_insts


def _patched_lower_ordered_insts(self, ordered):
    _split_excess_waits(self.nc, ordered)
    return _orig_lower_ordered_insts(self, ordered)


tile.TileContext._lower_ordered_insts = _patched_lower_ordered_insts


def _split_waits_drain_and_barrier(self, tick_clock, wait_clock):
    nc = self.nc
    probe = nc.sync.nop(nofuse=True)
    wait_clock.add_sem_waits(
        probe.ins, ScopedClock({None: tick_clock.global_clock}))
    si = probe.ins.sync_info
    waits = list(si.on_wait) if si is not None else []
    if len(waits) > _MAX_WAITS:
        si.on_wait = waits[:_MAX_WAITS]
        for i in range(_MAX_WAITS, len(waits), _MAX_WAITS):
            nxt = nc.sync.nop(nofuse=True)
            nxt.ins.sync_info = bass_rust.SyncInfo(
                on_wait=waits[i:i + _MAX_WAITS], on_update=[])
    nc.sync.drain()
    nc.all_engine_barrier()
    assert self.sems is not None
    popped = nc._tile_sem_poison_stack.pop()
    assert popped is self._sem_poison
    nc.clear_and_free_semaphores(list(self.sems.allocated().values()))
    nc.all_engine_barrier()


tile.TileContext._drain_and_barrier = _split_waits_drain_and_barrier

S, D, NCORES = 1536, 1024, 8
P_MAIN = 1024            # compacted+padded tokens per view
KT = D // 128            # 8 contraction tiles
TEMP_INV = 20.0          # 1 / 0.05
FP8_SCALE = 8.0          # f entries ~N(0, 1/32); x8 keeps them in e4m3's
                         # normal range (|f|*8 <~ 2, well under 240)
F32 = mybir.dt.float32
BF16 = mybir.dt.bfloat16
FP8 = mybir.dt.float8e4
AF = mybir.ActivationFunctionType
ALU = mybir.AluOpType
DR = mybir.MatmulPerfMode.DoubleRow


def _build(p: int, num_devices: int = NCORES) -> bass.Bass:
    PT = p // 128        # token tiles per view
    NB2 = 2 * PT         # block rows of the 2P x 2P matrix
    half = p // 512      # column strips per view
    exp_scale = TEMP_INV / (FP8_SCALE * FP8_SCALE)
    pos_scale = TEMP_INV / (FP8_SCALE * FP8_SCALE)

    nc = bass.Bass(num_devices=num_devices)
    h1 = nc.dram_tensor("h1", [p, D], F32, kind="ExternalInput")
    h2 = nc.dram_tensor("h2", [p, D], F32, kind="ExternalInput")
    # mask, pre-laid-out host-side as [128, PT] so token t = 128*col + row
    maskT = nc.dram_tensor("maskT", [128, PT], F32, kind="ExternalInput")
    out = nc.dram_tensor("loss", [1, 1], F32, kind="ExternalOutput")

    with tile.TileContext(nc) as tc, ExitStack() as ctx:
        const_pool = ctx.enter_context(tc.tile_pool(name="const", bufs=1))
        big = ctx.enter_context(tc.tile_pool(name="big", bufs=1))
        stat = ctx.enter_context(tc.tile_pool(name="stat", bufs=1))

        identB = const_pool.tile([128, 128], BF16)
        make_identity(nc, identB[:])
        ones_col = const_pool.tile([128, 1], F32)
        nc.gpsimd.memset(ones_col[:], 1.0)
        ones_sq = const_pool.tile([128, 128], F32)
        nc.gpsimd.memset(ones_sq[:], 1.0)
        ones_bf = const_pool.tile([128, 1], BF16)
        nc.gpsimd.memset(ones_bf[:], 1.0)
        one_f32 = const_pool.tile([1, 1], F32)
        nc.gpsimd.memset(one_f32[:], 1.0)
        msk = const_pool.tile([128, PT], F32)
        nc.sync.dma_start(msk[:], maskT[:])

        # tile-major transposed features: fT[:, t, k*128+c] holds
        # (8*f)^T[d = k*128 + partition, token = t*128 + c] in fp8e4, so
        # each token tile's transpose lands as one contiguous copy
        fT1 = big.tile([128, PT, KT * 128], FP8)
        fT2 = big.tile([128, PT, KT * 128], FP8)
        acc = stat.tile([128, NB2, 2 * half], F32)   # per-strip row sums
        poss20 = stat.tile([128, PT], F32)       # 20 * pos_sim
        csum_sb = stat.tile([1, 2 * p], F32)     # mirror column sums
        msk24 = stat.tile([128, NB2], F32)
        negK0 = stat.tile([128, 1], F32)
        recn = stat.tile([1, 1], F32)

        # zero row-sum slots never written (below-diagonal A/C strips)
        nc.gpsimd.memset(acc[:], 0.0)

        # ---- HAM warmup: keep the PE busy from t~0 so it upclocks.
        # The result is read and spilled to a DRAM scratch so dead-code
        # elimination cannot drop the matmuls.
        wdump = nc.dram_tensor("wdump", [1, 1], F32, kind="Internal")
        with tc.tile_pool(name="warm", bufs=1, space="PSUM") as wp:
            wps = wp.tile([128, 128], F32)
            for i in range(24):
                nc.tensor.matmul(wps[:], identB[:], identB[:],
                                 start=(i == 0), stop=(i == 23))
            wdbg = stat.tile([1, 1], F32)
            nc.vector.tensor_copy(wdbg[:], wps[0:1, 0:1])
            nc.sync.dma_start(wdump[:], wdbg[:])

        # ---- phase 0: mask-only precomputes ----
        with tc.tile_pool(name="ep0", bufs=1) as ep0, \
             tc.tile_pool(name="ep0_ps", bufs=1, space="PSUM") as ep0p:
            msum = ep0.tile([128, 1], F32)
            nc.vector.tensor_reduce(msum[:], msk[:],
                                    axis=mybir.AxisListType.X, op=ALU.add)
            nps = ep0p.tile([128, 1], F32)
            nc.tensor.matmul(nps[:], ones_sq[:], msum[:], start=True,
                             stop=True)
            # -K0 = 2n - 2P, broadcast to all partitions
            nc.scalar.activation(negK0[:], nps[:], AF.Copy, scale=2.0,
                                 bias=float(-2 * p))
            n2c = ep0.tile([1, 1], F32)
            nc.scalar.activation(n2c[:], nps[0:1, :], AF.Copy, scale=2.0)
            nc.vector.reciprocal(recn[:], n2c[:])   # 1/(2n)
            nc.vector.tensor_copy(msk24[:, 0:PT], msk[:])
            nc.vector.tensor_copy(msk24[:, PT:NB2], msk[:])

        with tc.tile_pool(name="mm_ps", bufs=2, space="PSUM") as mmp, \
             tc.tile_pool(name="cng_ps", bufs=1, space="PSUM") as cngp, \
             tc.tile_pool(name="es", bufs=4) as esp, \
             tc.tile_pool(name="ht", bufs=3) as htp, \
             tc.tile_pool(name="scr", bufs=2) as scr, \
             tc.tile_pool(name="sc", bufs=4) as scp, \
             tc.tile_pool(name="tt", bufs=2) as ttp:

            cng = cngp.tile([128, NB2], F32)     # mirror sums, token-major
            pending = []                         # deferred colsum matmuls

            def flush_pending():
                while pending:
                    pending.pop(0)()

            def emit_tile(v, t, src_dram, fT, tps):
                ht = htp.tile([128, D], F32, tag="ht", name=f"ht{v}_{t}")
                nc.sync.dma_start(ht[:], src_dram[t * 128:(t + 1) * 128, :])
                sq = scr.tile([128, D], BF16, tag="sq", name=f"sq{v}_{t}")
                ss = scp.tile([128, 1], F32, tag="ss", name=f"ss{v}_{t}")
                nc.vector.scalar_tensor_tensor(
                    out=sq[:], in0=ht[:], scalar=1.0, in1=ht[:],
                    op0=ALU.mult, op1=ALU.mult, accum_out=ss[:])
                # 1/sqrt(ss) = exp(-0.5*ln(ss)): Ln/Exp share a ScalarE
                # table with the strip exps, so no ACT_TABLE_LOAD thrash
                lnss = scp.tile([128, 1], F32, tag="ln", name=f"ln{v}_{t}")
                nc.scalar.activation(lnss[:], ss[:], AF.Ln)
                sc = scp.tile([128, 1], F32, tag="sc", name=f"sc{v}_{t}")
                nc.scalar.activation(sc[:], lnss[:], AF.Exp, scale=-0.5)
                scm = scp.tile([128, 1], F32, tag="scm", name=f"scm{v}_{t}")
                nc.vector.tensor_mul(scm[:], sc[:], msk[:, t:t + 1])
                fnb = scr.tile([128, D], BF16, tag="fn", name=f"fn{v}_{t}")
                nc.vector.tensor_scalar_mul(fnb[:], ht[:], scm[:])
                pt = tps.tile([128, D], BF16, tag="pt", name=f"pt{v}_{t}")
                for k in range(KT):
                    nc.tensor.transpose(pt[:, k * 128:(k + 1) * 128],
                                        fnb[:, k * 128:(k + 1) * 128],
                                        identB[:])
                # quantize to fp8 (x8) while moving PSUM->SBUF; the
                # tile-major fT makes this a contiguous copy
                nc.scalar.activation(fT[:, t, :], pt[:], AF.Copy,
                                     scale=FP8_SCALE)

            def emit_strip(r, csT, quad, cs_ps, first, last):
                """One sim strip at block-row r (global), local column
                strip csT of quadrant quad. A/C strips containing the
                diagonal are narrowed to skip fully-below-diagonal blocks."""
                lhsT = fT1 if r < PT else fT2
                rT = r % PT
                rhsT = fT1 if quad == "A" else fT2
                ko = 0                            # leading blocks skipped
                if quad != "B" and csT * 512 - rT * 128 < 128:
                    ko = rT - 4 * csT
                nw = 512 - 128 * ko               # strip width
                ps = mmp.tile([128, 512], F32, tag="ps",
                              name=f"ps{quad}_{csT}_{r}")
                rhs4 = rhsT[:, 4 * csT + ko:4 * csT + 4, :].rearrange(
                    "q t (k c) -> q k t c", k=KT)
                lhs3 = lhsT[:, rT, :].rearrange("q (k c) -> q k c", k=KT)
                for g in range(KT // 2):
                    nc.tensor.matmul(
                        ps[:, 0:nw],
                        lhs3[:, 2 * g:2 * g + 2, :],
                        rhs4[:, 2 * g:2 * g + 2, :, :],
                        perf_mode=DR,
                        start=(g == 0), stop=(g == KT // 2 - 1))
                # previous strip's colsum lands here: by now its exp result
                # is ready, so the PE doesn't stall on it
                flush_pending()
                es = esp.tile([128, 512], BF16, tag="es",
                              name=f"es{quad}_{csT}_{r}")
                cs_g = csT if quad == "A" else half + csT
                if quad == "B":
                    nc.scalar.activation(es[:], ps[:], AF.Exp,
                                         scale=exp_scale,
                                         accum_out=acc[:, r, cs_g:cs_g + 1])
                    if csT * 4 <= rT <= csT * 4 + 3:
                        # pos_sim lives on this strip's diagonal block:
                        # extract it exactly from the f32 PSUM sims
                        jb = rT - csT * 4
                        sct = ttp.tile([128, 128], F32, tag="sct",
                                       name=f"sct_{r}")
                        nc.vector.scalar_tensor_tensor(
                            out=sct[:],
                            in0=ps[:, jb * 128:(jb + 1) * 128],
                            scalar=pos_scale,
                            in1=identB[:],
                            op0=ALU.mult,
                            op1=ALU.mult,
                            accum_out=poss20[:, rT:rT + 1])
                elif ko == 0 and csT * 512 - rT * 128 >= 128:
                    # strictly above the diagonal: plain exp + row sums
                    nc.scalar.activation(
                        es[:], ps[:], AF.Exp, scale=exp_scale,
                        accum_out=acc[:, r, cs_g:cs_g + 1])
                else:
                    # first block is the diagonal one: strict upper keep
                    nc.scalar.activation(es[:, 0:nw], ps[:, 0:nw], AF.Exp,
                                         scale=exp_scale)
                    # keep col > row: -1 + (-1)*p + 1*c >= 0
                    nc.gpsimd.affine_select(
                        out=es[:, 0:nw], in_=es[:, 0:nw],
                        compare_op=ALU.is_ge,
                        fill=0.0, base=-1, pattern=[[1, nw]],
                        channel_multiplier=-1)
                    nc.vector.tensor_reduce(acc[:, r, cs_g:cs_g + 1],
                                            es[:, 0:nw],
                                            axis=mybir.AxisListType.X,
                                            op=ALU.add)

                def colsum(es=es, first=first, last=last, vec=cs_ps,
                           ko=ko, nw=nw):
                    nc.tensor.matmul(vec[0:1, ko * 128:ko * 128 + nw],
                                     ones_bf[:], es[:, 0:nw],
                                     start=first, stop=last,
                                     skip_group_check=True)
                pending.append(colsum)

            def emit_mirror(base_chunk, nchunks):
                # transpose csum_sb chunks to token-major via K=1 matmuls
                for c in range(base_chunk, base_chunk + nchunks):
                    nc.tensor.matmul(cng[:, c:c + 1],
                                     csum_sb[0:1, c * 128:(c + 1) * 128],
                                     one_f32[:], start=True, stop=True)

            def a_strips(csT):
                return [(r, "A") for r in range(min(4 * csT + 4, PT))]

            def bc_strips(csT):
                # C diag strips first (their exp->affine->reduce chain
                # overlaps later strips; the emission leader is full
                # width, as the PSUM colsum accumulation group needs),
                # pure-upper C next, B strips last so the drain tail
                # only needs cheap exp+accum
                crows = list(range(min(4 * csT + 4, PT)))
                diag = [(PT + rc, "C") for rc in crows
                        if csT * 512 - rc * 128 < 128]
                pure = [(PT + rc, "C") for rc in crows
                        if csT * 512 - rc * 128 >= 128]
                return diag + pure + [(r, "B") for r in range(PT)]

            # ---- merged schedule: all 16 tiles in sequence, ready
            # strips spread across the remaining tile slots so the PE
            # never starves while tiles DMA/normalize/transpose ----
            with tc.tile_pool(name="tp_ps", bufs=1, space="PSUM") as tps, \
                 tc.tile_pool(name="psA", bufs=1, space="PSUM") as psAp, \
                 tc.tile_pool(name="psB", bufs=1, space="PSUM") as psBp:
                psA = [psAp.tile([1, 512], F32, name=f"psA{c}")
                       for c in range(half)]
                psB = [psBp.tile([1, 512], F32, name=f"psB{c}")
                       for c in range(half)]
                # group bookkeeping: when the last colsum of a column
                # group has been emitted, immediately fold its [1,512]
                # accumulator to token-major mirror columns
                remaining = {}
                mirror_pending = []   # [countdown, closure]

                def tick_mirrors():
                    for ent in list(mirror_pending):
                        ent[0] -= 1
                        if ent[0] <= 0:
                            mirror_pending.remove(ent)
                            ent[1]()

                def finish_group(v, csT):
                    flush_pending()
                    vec = (psA if v == 1 else psB)[csT]
                    base = 0 if v == 1 else p
                    nc.vector.tensor_copy(
                        csum_sb[0:1, base + csT * 512:base + (csT + 1) * 512],
                        vec[:])
                    chunk0 = (0 if v == 1 else PT) + 4 * csT
                    # the K=1 fold matmuls wait on the DVE copy above;
                    # defer them a couple of strips so the PE never stalls
                    mirror_pending.append(
                        [2, lambda c=chunk0: emit_mirror(c, 4)])

                def do_strip(s):
                    v, csT = (1 if s[2] == "A" else 2), s[1]
                    emit_strip(s[0], s[1], s[2], s[3], s[4], s[5])
                    tick_mirrors()
                    remaining[(v, csT)] -= 1
                    if remaining[(v, csT)] == 0:
                        finish_group(v, csT)

                queue = []
                tiles = [(1, t, h1, fT1) for t in range(PT)] + \
                        [(2, t, h2, fT2) for t in range(PT)]
                for i, (v, t, srcd, fT) in enumerate(tiles):
                    emit_tile(v, t, srcd, fT, tps)
                    flush_pending()
                    if t % 4 == 3:
                        csT = t // 4
                        group = a_strips(csT) if v == 1 else bc_strips(csT)
                        vecs = psA if v == 1 else psB
                        remaining[(v, csT)] = len(group)
                        for j, (r, qd) in enumerate(group):
                            queue.append((r, csT, qd, vecs[csT],
                                          j == 0, j == len(group) - 1))
                    tiles_left = len(tiles) - 1 - i
                    if tiles_left > 0 and queue:
                        n_emit = -(-len(queue) // (tiles_left + 1))
                        for _ in range(n_emit):
                            do_strip(queue.pop(0))
                for s in queue:
                    do_strip(s)
                flush_pending()
                for ent in mirror_pending:
                    ent[1]()
                mirror_pending.clear()

            # ---- epilogue: final reduction chain ----
            with tc.tile_pool(name="ep", bufs=1) as ep, \
                 tc.tile_pool(name="ep_ps", bufs=1, space="PSUM") as epp:
                cngs = ep.tile([128, NB2], F32)
                nc.vector.tensor_copy(cngs[:], cng[:])
                ng = ep.tile([128, NB2], F32)
                nc.vector.tensor_reduce(ng[:], acc[:],
                                        axis=mybir.AxisListType.X,
                                        op=ALU.add)
                den = ep.tile([128, NB2], F32)
                nc.vector.tensor_add(den[:], ng[:], cngs[:])
                nc.vector.tensor_scalar_add(den[:], den[:], negK0[:])
                lg = ep.tile([128, NB2], F32)
                nc.scalar.activation(lg[:], den[:], AF.Ln)
                pm = ep.tile([128, NB2], F32)
                nc.vector.tensor_copy(pm[:, 0:PT], poss20[:])
                nc.vector.tensor_copy(pm[:, PT:NB2], poss20[:])
                d1 = ep.tile([128, NB2], F32)
                nc.vector.tensor_sub(d1[:], lg[:], pm[:])
                ptok = ep.tile([128, NB2], F32)
                tsum = ep.tile([128, 1], F32)
                nc.vector.scalar_tensor_tensor(
                    out=ptok[:], in0=d1[:], scalar=1.0, in1=msk24[:],
                    op0=ALU.mult, op1=ALU.mult, accum_out=tsum[:])
                lps = epp.tile([1, 1], F32)
                nc.tensor.matmul(lps[:], ones_col[:], tsum[:], start=True,
                                 stop=True)
                lsb = ep.tile([1, 1], F32)
                nc.vector.tensor_mul(lsb[:], lps[:], recn[:])
                nc.sync.dma_start(out[:], lsb[:])

    return nc


_NC = {}


def _get_nc(p: int) -> bass.Bass:
    if p not in _NC:
        _NC[p] = _build(p)
    return _NC[p]


def _mask_layout(mask_col: np.ndarray, p: int) -> np.ndarray:
    # token t = 128 * col + row  ->  [128, PT]
    return np.ascontiguousarray(
        mask_col.astype(np.float32).reshape(p // 128, 128).T)


def _in_maps(h1, h2, mask, p):
    maps = []
    for b in range(NCORES):
        idx = np.argsort(~mask[b], kind="stable")[:p]
        maps.append({
            "h1": np.ascontiguousarray(h1[b][idx]),
            "h2": np.ascontiguousarray(h2[b][idx]),
            "maskT": _mask_layout(mask[b][idx], p),
        })
    return maps


def kernel(last_hidden_states_1, last_hidden_states_2, token_mask_batch):
    h1 = np.ascontiguousarray(np.asarray(last_hidden_states_1,
                                         dtype=np.float32))
    h2 = np.ascontiguousarray(np.asarray(last_hidden_states_2,
                                         dtype=np.float32))
    mask = np.asarray(token_mask_batch).astype(bool)
    assert h1.shape == (NCORES, S, D), h1.shape

    p = P_MAIN if int(mask.sum(axis=1).max()) <= P_MAIN else S
    nc = _get_nc(p)
    res = run_bass_kernel_spmd(nc, _in_maps(h1, h2, mask, p),
                               list(range(NCORES)))
    vals = [np.asarray(res.results[b]["loss"], dtype=np.float32).reshape(())
            for b in range(NCORES)]
    return np.float32(np.mean(vals))


# revision 23
# speedup vs baseline: 1.0483x; 1.0148x over previous
"""ContraCLM token-level contrastive loss on 8 Trainium2 NeuronCores.

Data-parallel over the batch: core b handles sample b (B=8). Host-side,
each sample's unmasked tokens are compacted to the front (a pure gather /
layout transform; the kernel still sees real data rows for pads so norms
never hit 0/0) and padded to P=1024 (n ~ Binomial(1536, .5) ~ 768; the
build is generic in P with a P=1536 fallback if some n > 1024).

Per core, with P=1024, D=1024, T=0.05:

  f_v = l2norm(h_v) with pad rows zeroed (mask folded into the reciprocal
  norm scale); G_v = (8*f_v)^T stored [D, P] in fp8e4 (x8 keeps entries
  in e4m3's normal range).

  The 2P x 2P similarity matrix [[A B];[B^T C]] (A = f1 f1^T etc.) is
  symmetric, so only A/C upper-triangle strips and all of B are computed
  as [128, 512] PSUM strips (fp8 DoubleRow, K=1024). exp(sim/T) row sums
  come from the ScalarE activation free-dim accumulator; strips containing
  the diagonal get a strict-upper affine_select then a DVE row-sum. The
  mirrored (lower-triangle) contributions are recovered from column sums:
  a ones-vector stationary matmul streams each es strip into a per-column
  [1, 512] PSUM accumulator, which is transposed to token-major layout at
  the end via K=1 outer-product matmuls.

  B's diagonal is exp(pos_sim/T): it is left inside the row/col sums
  (denominator = Ng + pos exactly), and 20*pos_sim for the numerator is
  extracted exactly from the f32 PSUM sim diagonal with a fused
  tensor_tensor_reduce against an identity tile.

  Pad columns contribute exp(0)=1 to every row sum: subtract
  K0 = 2P - 2n. per_tok = ln(denom) - 20*pos_sim, masked mean over 2n
  tokens; per-sample means are averaged on the host (no collective).
"""

import sys

for _p in ("/opt/trn_rl_repo", "/opt/pypackages"):
    if _p not in sys.path:
        sys.path.append(_p)

from contextlib import ExitStack

import numpy as np

import bass_rust

import concourse.bass as bass
import concourse.tile as tile
from concourse import mybir
from concourse.bass_utils import run_bass_kernel_spmd
from concourse.masks import make_identity
from concourse.vector_clock import ScopedClock

# The walrus build in this container encodes at most 2 sync waits per
# instruction (bass_rust's inst_waits_full agrees), but Tile's semaphore
# assignment can attach more. Hoist excess waits onto unfusable same-engine
# NoOps immediately before the instruction — the engine executes its queue
# in order, so semantics are preserved.
_MAX_WAITS = 1


def _split_excess_waits(nc, ordered):
    for bb_name, insts in ordered.items():
        out = []
        changed = False
        for inst in insts:
            si = getattr(inst, "sync_info", None)
            waits = list(si.on_wait) if si is not None else []
            if len(waits) > _MAX_WAITS:
                changed = True
                extra, keep = waits[:-_MAX_WAITS], waits[-_MAX_WAITS:]
                for i in range(0, len(extra), _MAX_WAITS):
                    out.append(mybir.InstNoOp(
                        name=nc.get_next_instruction_name(),
                        sync_info=mybir.SyncInfo(
                            on_wait=extra[i:i + _MAX_WAITS], on_update=[]),
                        bass_nofuse=True,
                        engine=inst.engine,
                    ))
                si.on_wait = keep
            out.append(inst)
        if changed:
            insts[:] = out


_orig_lower_ordered_insts = tile.TileContext._lower_ordered_insts


def _patched_lower_ordered_insts(self, ordered):
    _split_excess_waits(self.nc, ordered)
    return _orig_lower_ordered_insts(self, ordered)


tile.TileContext._lower_ordered_insts = _patched_lower_ordered_insts


def _split_waits_drain_and_barrier(self, tick_clock, wait_clock):
    nc = self.nc
    probe = nc.sync.nop(nofuse=True)
    wait_clock.add_sem_waits(
        probe.ins, ScopedClock({None: tick_clock.global_clock}))
    si = probe.ins.sync_info
    waits = list(si.on_wait) if si is not None else []
    if len(waits) > _MAX_WAITS:
        si.on_wait = waits[:_MAX_WAITS]
        for i in range(_MAX_WAITS, len(waits), _MAX_WAITS):
            nxt = nc.sync.nop(nofuse=True)
            nxt.ins.sync_info = bass_rust.SyncInfo(
                on_wait=waits[i:i + _MAX_WAITS], on_update=[])
    nc.sync.drain()
    nc.all_engine_barrier()
    assert self.sems is not None
    popped = nc._tile_sem_poison_stack.pop()
    assert popped is self._sem_poison
    nc.clear_and_free_semaphores(list(self.sems.allocated().values()))
    nc.all_engine_barrier()


tile.TileContext._drain_and_barrier = _split_waits_drain_and_barrier

S, D, NCORES = 1536, 1024, 8
P_MAIN = 1024            # compacted+padded tokens per view
KT = D // 128            # 8 contraction tiles
TEMP_INV = 20.0          # 1 / 0.05
FP8_SCALE = 8.0          # f entries ~N(0, 1/32); x8 keeps them in e4m3's
                         # normal range (|f|*8 <~ 2, well under 240)
F32 = mybir.dt.float32
BF16 = mybir.dt.bfloat16
FP8 = mybir.dt.float8e4
AF = mybir.ActivationFunctionType
ALU = mybir.AluOpType
DR = mybir.MatmulPerfMode.DoubleRow


def _build(p: int, num_devices: int = NCORES) -> bass.Bass:
    PT = p // 128        # token tiles per view
    NB2 = 2 * PT         # block rows of the 2P x 2P matrix
    half = p // 512      # column strips per view
    exp_scale = TEMP_INV / (FP8_SCALE * FP8_SCALE)
    pos_scale = TEMP_INV / (FP8_SCALE * FP8_SCALE)

    nc = bass.Bass(num_devices=num_devices)
    h1 = nc.dram_tensor("h1", [p, D], F32, kind="ExternalInput")
    h2 = nc.dram_tensor("h2", [p, D], F32, kind="ExternalInput")
    # mask, pre-laid-out host-side as [128, PT] so token t = 128*col + row
    maskT = nc.dram_tensor("maskT", [128, PT], F32, kind="ExternalInput")
    out = nc.dram_tensor("loss", [1, 1], F32, kind="ExternalOutput")

    with tile.TileContext(nc) as tc, ExitStack() as ctx:
        const_pool = ctx.enter_context(tc.tile_pool(name="const", bufs=1))
        big = ctx.enter_context(tc.tile_pool(name="big", bufs=1))
        stat = ctx.enter_context(tc.tile_pool(name="stat", bufs=1))

        identB = const_pool.tile([128, 128], BF16)
        make_identity(nc, identB[:])
        ones_col = const_pool.tile([128, 1], F32)
        nc.gpsimd.memset(ones_col[:], 1.0)
        ones_sq = const_pool.tile([128, 128], F32)
        nc.gpsimd.memset(ones_sq[:], 1.0)
        ones_bf = const_pool.tile([128, 1], BF16)
        nc.gpsimd.memset(ones_bf[:], 1.0)
        one_f32 = const_pool.tile([1, 1], F32)
        nc.gpsimd.memset(one_f32[:], 1.0)
        msk = const_pool.tile([128, PT], F32)
        nc.sync.dma_start(msk[:], maskT[:])

        # tile-major transposed features: fT[:, t, k*128+c] holds
        # (8*f)^T[d = k*128 + partition, token = t*128 + c] in fp8e4, so
        # each token tile's transpose lands as one contiguous copy
        fT1 = big.tile([128, PT, KT * 128], FP8)
        fT2 = big.tile([128, PT, KT * 128], FP8)
        acc = stat.tile([128, NB2, 2 * half], F32)   # per-strip row sums
        poss20 = stat.tile([128, PT], F32)       # 20 * pos_sim
        csum_sb = stat.tile([1, 2 * p], F32)     # mirror column sums
        msk24 = stat.tile([128, NB2], F32)
        negK0 = stat.tile([128, 1], F32)
        recn = stat.tile([1, 1], F32)

        # zero row-sum slots never written (below-diagonal A/C strips)
        nc.gpsimd.memset(acc[:], 0.0)

        # ---- HAM warmup: keep the PE busy from t~0 so it upclocks.
        # The result is read and spilled to a DRAM scratch so dead-code
        # elimination cannot drop the matmuls.
        wdump = nc.dram_tensor("wdump", [1, 1], F32, kind="Internal")
        with tc.tile_pool(name="warm", bufs=1, space="PSUM") as wp:
            wps = wp.tile([128, 128], F32)
            for i in range(24):
                nc.tensor.matmul(wps[:], identB[:], identB[:],
                                 start=(i == 0), stop=(i == 23))
            wdbg = stat.tile([1, 1], F32)
            nc.vector.tensor_copy(wdbg[:], wps[0:1, 0:1])
            nc.sync.dma_start(wdump[:], wdbg[:])

        # ---- phase 0: mask-only precomputes ----
        with tc.tile_pool(name="ep0", bufs=1) as ep0, \
             tc.tile_pool(name="ep0_ps", bufs=1, space="PSUM") as ep0p:
            msum = ep0.tile([128, 1], F32)
            nc.vector.tensor_reduce(msum[:], msk[:],
                                    axis=mybir.AxisListType.X, op=ALU.add)
            nps = ep0p.tile([128, 1], F32)
            nc.tensor.matmul(nps[:], ones_sq[:], msum[:], start=True,
                             stop=True)
            # -K0 = 2n - 2P, broadcast to all partitions
            nc.scalar.activation(negK0[:], nps[:], AF.Copy, scale=2.0,
                                 bias=float(-2 * p))
            n2c = ep0.tile([1, 1], F32)
            nc.scalar.activation(n2c[:], nps[0:1, :], AF.Copy, scale=2.0)
            nc.vector.reciprocal(recn[:], n2c[:])   # 1/(2n)
            nc.vector.tensor_copy(msk24[:, 0:PT], msk[:])
            nc.vector.tensor_copy(msk24[:, PT:NB2], msk[:])

        with tc.tile_pool(name="mm_ps", bufs=2, space="PSUM") as mmp, \
             tc.tile_pool(name="cng_ps", bufs=1, space="PSUM") as cngp, \
             tc.tile_pool(name="es", bufs=4) as esp, \
             tc.tile_pool(name="ht", bufs=3) as htp, \
             tc.tile_pool(name="scr", bufs=2) as scr, \
             tc.tile_pool(name="sc", bufs=4) as scp, \
             tc.tile_pool(name="tt", bufs=2) as ttp:

            cng = cngp.tile([128, NB2], F32)     # mirror sums, token-major
            pending = []                         # deferred colsum matmuls

            def flush_pending():
                while pending:
                    pending.pop(0)()

            def emit_tile(v, t, src_dram, fT, tps):
                ht = htp.tile([128, D], F32, tag="ht", name=f"ht{v}_{t}")
                nc.sync.dma_start(ht[:], src_dram[t * 128:(t + 1) * 128, :])
                sq = scr.tile([128, D], BF16, tag="sq", name=f"sq{v}_{t}")
                ss = scp.tile([128, 1], F32, tag="ss", name=f"ss{v}_{t}")
                nc.vector.scalar_tensor_tensor(
                    out=sq[:], in0=ht[:], scalar=1.0, in1=ht[:],
                    op0=ALU.mult, op1=ALU.mult, accum_out=ss[:])
                # 1/sqrt(ss) = exp(-0.5*ln(ss)): Ln/Exp share a ScalarE
                # table with the strip exps, so no ACT_TABLE_LOAD thrash
                lnss = scp.tile([128, 1], F32, tag="ln", name=f"ln{v}_{t}")
                nc.scalar.activation(lnss[:], ss[:], AF.Ln)
                sc = scp.tile([128, 1], F32, tag="sc", name=f"sc{v}_{t}")
                nc.scalar.activation(sc[:], lnss[:], AF.Exp, scale=-0.5)
                scm = scp.tile([128, 1], F32, tag="scm", name=f"scm{v}_{t}")
                nc.vector.tensor_mul(scm[:], sc[:], msk[:, t:t + 1])
                fnb = scr.tile([128, D], BF16, tag="fn", name=f"fn{v}_{t}")
                nc.vector.tensor_scalar_mul(fnb[:], ht[:], scm[:])
                pt = tps.tile([128, D], BF16, tag="pt", name=f"pt{v}_{t}")
                for k in range(KT):
                    nc.tensor.transpose(pt[:, k * 128:(k + 1) * 128],
                                        fnb[:, k * 128:(k + 1) * 128],
                                        identB[:])
                # quantize to fp8 (x8) while moving PSUM->SBUF; the
                # tile-major fT makes this a contiguous copy
                nc.scalar.activation(fT[:, t, :], pt[:], AF.Copy,
                                     scale=FP8_SCALE)

            def emit_strip(r, csT, quad, cs_ps, first, last):
                """One sim strip at block-row r (global), local column
                strip csT of quadrant quad. A/C strips containing the
                diagonal are narrowed to skip fully-below-diagonal blocks."""
                lhsT = fT1 if r < PT else fT2
                rT = r % PT
                rhsT = fT1 if quad == "A" else fT2
                ko = 0                            # leading blocks skipped
                if quad != "B" and csT * 512 - rT * 128 < 128:
                    ko = rT - 4 * csT
                nw = 512 - 128 * ko               # strip width
                ps = mmp.tile([128, 512], F32, tag="ps",
                              name=f"ps{quad}_{csT}_{r}")
                rhs4 = rhsT[:, 4 * csT + ko:4 * csT + 4, :].rearrange(
                    "q t (k c) -> q k t c", k=KT)
                lhs3 = lhsT[:, rT, :].rearrange("q (k c) -> q k c", k=KT)
                for g in range(KT // 2):
                    nc.tensor.matmul(
                        ps[:, 0:nw],
                        lhs3[:, 2 * g:2 * g + 2, :],
                        rhs4[:, 2 * g:2 * g + 2, :, :],
                        perf_mode=DR,
                        start=(g == 0), stop=(g == KT // 2 - 1))
                # previous strip's colsum lands here: by now its exp result
                # is ready, so the PE doesn't stall on it
                flush_pending()
                es = esp.tile([128, 512], BF16, tag="es",
                              name=f"es{quad}_{csT}_{r}")
                cs_g = csT if quad == "A" else half + csT
                if quad == "B":
                    nc.scalar.activation(es[:], ps[:], AF.Exp,
                                         scale=exp_scale,
                                         accum_out=acc[:, r, cs_g:cs_g + 1])
                    if csT * 4 <= rT <= csT * 4 + 3:
                        # pos_sim lives on this strip's diagonal block:
                        # extract it exactly from the f32 PSUM sims
                        jb = rT - csT * 4
                        sct = ttp.tile([128, 128], F32, tag="sct",
                                       name=f"sct_{r}")
                        nc.vector.scalar_tensor_tensor(
                            out=sct[:],
                            in0=ps[:, jb * 128:(jb + 1) * 128],
                            scalar=pos_scale,
                            in1=identB[:],
                            op0=ALU.mult,
                            op1=ALU.mult,
                            accum_out=poss20[:, rT:rT + 1])
                elif ko == 0 and csT * 512 - rT * 128 >= 128:
                    # strictly above the diagonal: plain exp + row sums
                    nc.scalar.activation(
                        es[:], ps[:], AF.Exp, scale=exp_scale,
                        accum_out=acc[:, r, cs_g:cs_g + 1])
                else:
                    # first block is the diagonal one: strict upper keep
                    nc.scalar.activation(es[:, 0:nw], ps[:, 0:nw], AF.Exp,
                                         scale=exp_scale)
                    # keep col > row: -1 + (-1)*p + 1*c >= 0
                    nc.gpsimd.affine_select(
                        out=es[:, 0:nw], in_=es[:, 0:nw],
                        compare_op=ALU.is_ge,
                        fill=0.0, base=-1, pattern=[[1, nw]],
                        channel_multiplier=-1)
                    nc.vector.tensor_reduce(acc[:, r, cs_g:cs_g + 1],
                                            es[:, 0:nw],
                                            axis=mybir.AxisListType.X,
                                            op=ALU.add)

                def colsum(es=es, first=first, last=last, vec=cs_ps,
                           ko=ko, nw=nw):
                    nc.tensor.matmul(vec[0:1, ko * 128:ko * 128 + nw],
                                     ones_bf[:], es[:, 0:nw],
                                     start=first, stop=last,
                                     skip_group_check=True)
                pending.append(colsum)

            def emit_mirror(base_chunk, nchunks):
                # transpose csum_sb chunks to token-major via K=1 matmuls
                for c in range(base_chunk, base_chunk + nchunks):
                    nc.tensor.matmul(cng[:, c:c + 1],
                                     csum_sb[0:1, c * 128:(c + 1) * 128],
                                     one_f32[:], start=True, stop=True)

            def a_strips(csT):
                return [(r, "A") for r in range(min(4 * csT + 4, PT))]

            def bc_strips(csT):
                # C diag strips first (their exp->affine->reduce chain
                # overlaps later strips; the emission leader is full
                # width, as the PSUM colsum accumulation group needs),
                # pure-upper C next, B strips last so the drain tail
                # only needs cheap exp+accum
                crows = list(range(min(4 * csT + 4, PT)))
                diag = [(PT + rc, "C") for rc in crows
                        if csT * 512 - rc * 128 < 128]
                pure = [(PT + rc, "C") for rc in crows
                        if csT * 512 - rc * 128 >= 128]
                return diag + pure + [(r, "B") for r in range(PT)]

            # ---- merged schedule: all 16 tiles in sequence, ready
            # strips spread across the remaining tile slots so the PE
            # never starves while tiles DMA/normalize/transpose ----
            with tc.tile_pool(name="tp_ps", bufs=1, space="PSUM") as tps, \
                 tc.tile_pool(name="psA", bufs=1, space="PSUM") as psAp, \
                 tc.tile_pool(name="psB", bufs=1, space="PSUM") as psBp:
                psA = [psAp.tile([1, 512], F32, name=f"psA{c}")
                       for c in range(half)]
                psB = [psBp.tile([1, 512], F32, name=f"psB{c}")
                       for c in range(half)]
                # group bookkeeping: when the last colsum of a column
                # group has been emitted, immediately fold its [1,512]
                # accumulator to token-major mirror columns
                n_a_total = sum(len(a_strips(c)) for c in range(half))
                a_done = 0

                def finish_a():
                    # A colsum groups complete: fold them to token-major
                    flush_pending()
                    for c in range(half):
                        nc.vector.tensor_copy(
                            csum_sb[0:1, c * 512:(c + 1) * 512], psA[c][:])
                    emit_mirror(0, PT)

                def do_strip(s):
                    nonlocal a_done
                    emit_strip(s[0], s[1], s[2], s[3], s[4], s[5])
                    if s[2] == "A":
                        a_done += 1
                        if a_done == n_a_total:
                            finish_a()

                queue = []
                tiles = [(1, t, h1, fT1) for t in range(PT)] + \
                        [(2, t, h2, fT2) for t in range(PT)]
                for i, (v, t, srcd, fT) in enumerate(tiles):
                    emit_tile(v, t, srcd, fT, tps)
                    flush_pending()
                    if t % 4 == 3:
                        csT = t // 4
                        group = a_strips(csT) if v == 1 else bc_strips(csT)
                        vecs = psA if v == 1 else psB
                        for j, (r, qd) in enumerate(group):
                            queue.append((r, csT, qd, vecs[csT],
                                          j == 0, j == len(group) - 1))
                    tiles_left = len(tiles) - 1 - i
                    if tiles_left > 0 and queue:
                        n_emit = -(-len(queue) // (tiles_left + 1))
                        for _ in range(n_emit):
                            do_strip(queue.pop(0))
                for s in queue:
                    do_strip(s)
                flush_pending()
                for c in range(half):
                    nc.vector.tensor_copy(
                        csum_sb[0:1, p + c * 512:p + (c + 1) * 512],
                        psB[c][:])
            emit_mirror(PT, PT)

            # ---- epilogue: final reduction chain ----
            with tc.tile_pool(name="ep", bufs=1) as ep, \
                 tc.tile_pool(name="ep_ps", bufs=1, space="PSUM") as epp:
                cngs = ep.tile([128, NB2], F32)
                nc.vector.tensor_copy(cngs[:], cng[:])
                ng = ep.tile([128, NB2], F32)
                nc.vector.tensor_reduce(ng[:], acc[:],
                                        axis=mybir.AxisListType.X,
                                        op=ALU.add)
                den = ep.tile([128, NB2], F32)
                nc.vector.tensor_add(den[:], ng[:], cngs[:])
                nc.vector.tensor_scalar_add(den[:], den[:], negK0[:])
                lg = ep.tile([128, NB2], F32)
                nc.scalar.activation(lg[:], den[:], AF.Ln)
                pm = ep.tile([128, NB2], F32)
                nc.vector.tensor_copy(pm[:, 0:PT], poss20[:])
                nc.vector.tensor_copy(pm[:, PT:NB2], poss20[:])
                d1 = ep.tile([128, NB2], F32)
                nc.vector.tensor_sub(d1[:], lg[:], pm[:])
                ptok = ep.tile([128, NB2], F32)
                tsum = ep.tile([128, 1], F32)
                nc.vector.scalar_tensor_tensor(
                    out=ptok[:], in0=d1[:], scalar=1.0, in1=msk24[:],
                    op0=ALU.mult, op1=ALU.mult, accum_out=tsum[:])
                lps = epp.tile([1, 1], F32)
                nc.tensor.matmul(lps[:], ones_col[:], tsum[:], start=True,
                                 stop=True)
                lsb = ep.tile([1, 1], F32)
                nc.vector.tensor_mul(lsb[:], lps[:], recn[:])
                nc.sync.dma_start(out[:], lsb[:])

    return nc


_NC = {}


def _get_nc(p: int) -> bass.Bass:
    if p not in _NC:
        _NC[p] = _build(p)
    return _NC[p]


def _mask_layout(mask_col: np.ndarray, p: int) -> np.ndarray:
    # token t = 128 * col + row  ->  [128, PT]
    return np.ascontiguousarray(
        mask_col.astype(np.float32).reshape(p // 128, 128).T)


def _in_maps(h1, h2, mask, p):
    maps = []
    for b in range(NCORES):
        idx = np.argsort(~mask[b], kind="stable")[:p]
        maps.append({
            "h1": np.ascontiguousarray(h1[b][idx]),
            "h2": np.ascontiguousarray(h2[b][idx]),
            "maskT": _mask_layout(mask[b][idx], p),
        })
    return maps


def kernel(last_hidden_states_1, last_hidden_states_2, token_mask_batch):
    h1 = np.ascontiguousarray(np.asarray(last_hidden_states_1,
                                         dtype=np.float32))
    h2 = np.ascontiguousarray(np.asarray(last_hidden_states_2,
                                         dtype=np.float32))
    mask = np.asarray(token_mask_batch).astype(bool)
    assert h1.shape == (NCORES, S, D), h1.shape

    p = P_MAIN if int(mask.sum(axis=1).max()) <= P_MAIN else S
    nc = _get_nc(p)
    res = run_bass_kernel_spmd(nc, _in_maps(h1, h2, mask, p),
                               list(range(NCORES)))
    vals = [np.asarray(res.results[b]["loss"], dtype=np.float32).reshape(())
            for b in range(NCORES)]
    return np.float32(np.mean(vals))


# revision 24
# speedup vs baseline: 1.0721x; 1.0227x over previous
"""ContraCLM token-level contrastive loss on 8 Trainium2 NeuronCores.

Data-parallel over the batch: core b handles sample b (B=8). Host-side,
each sample's unmasked tokens are compacted to the front (a pure gather /
layout transform; the kernel still sees real data rows for pads so norms
never hit 0/0) and padded to P=1024 (n ~ Binomial(1536, .5) ~ 768; the
build is generic in P with a P=1536 fallback if some n > 1024).

Per core, with P=1024, D=1024, T=0.05:

  f_v = l2norm(h_v) with pad rows zeroed (mask folded into the reciprocal
  norm scale); G_v = (8*f_v)^T stored [D, P] in fp8e4 (x8 keeps entries
  in e4m3's normal range).

  The 2P x 2P similarity matrix [[A B];[B^T C]] (A = f1 f1^T etc.) is
  symmetric, so only A/C upper-triangle strips and all of B are computed
  as [128, 512] PSUM strips (fp8 DoubleRow, K=1024). exp(sim/T) row sums
  come from the ScalarE activation free-dim accumulator; strips containing
  the diagonal get a strict-upper affine_select then a DVE row-sum. The
  mirrored (lower-triangle) contributions are recovered from column sums:
  a ones-vector stationary matmul streams each es strip into a per-column
  [1, 512] PSUM accumulator, which is transposed to token-major layout at
  the end via K=1 outer-product matmuls.

  B's diagonal is exp(pos_sim/T): it is left inside the row/col sums
  (denominator = Ng + pos exactly), and 20*pos_sim for the numerator is
  extracted exactly from the f32 PSUM sim diagonal with a fused
  tensor_tensor_reduce against an identity tile.

  Pad columns contribute exp(0)=1 to every row sum: subtract
  K0 = 2P - 2n. per_tok = ln(denom) - 20*pos_sim, masked mean over 2n
  tokens; per-sample means are averaged on the host (no collective).
"""

import sys

for _p in ("/opt/trn_rl_repo", "/opt/pypackages"):
    if _p not in sys.path:
        sys.path.append(_p)

from contextlib import ExitStack

import numpy as np

import bass_rust

import concourse.bass as bass
import concourse.tile as tile
from concourse import mybir
from concourse.bass_utils import run_bass_kernel_spmd
from concourse.masks import make_identity
from concourse.vector_clock import ScopedClock

# The walrus build in this container encodes at most 2 sync waits per
# instruction (bass_rust's inst_waits_full agrees), but Tile's semaphore
# assignment can attach more. Hoist excess waits onto unfusable same-engine
# NoOps immediately before the instruction — the engine executes its queue
# in order, so semantics are preserved.
_MAX_WAITS = 1


def _split_excess_waits(nc, ordered):
    for bb_name, insts in ordered.items():
        out = []
        changed = False
        for inst in insts:
            si = getattr(inst, "sync_info", None)
            waits = list(si.on_wait) if si is not None else []
            if len(waits) > _MAX_WAITS:
                changed = True
                extra, keep = waits[:-_MAX_WAITS], waits[-_MAX_WAITS:]
                for i in range(0, len(extra), _MAX_WAITS):
                    out.append(mybir.InstNoOp(
                        name=nc.get_next_instruction_name(),
                        sync_info=mybir.SyncInfo(
                            on_wait=extra[i:i + _MAX_WAITS], on_update=[]),
                        bass_nofuse=True,
                        engine=inst.engine,
                    ))
                si.on_wait = keep
            out.append(inst)
        if changed:
            insts[:] = out


_orig_lower_ordered_insts = tile.TileContext._lower_ordered_insts


def _patched_lower_ordered_insts(self, ordered):
    _split_excess_waits(self.nc, ordered)
    return _orig_lower_ordered_insts(self, ordered)


tile.TileContext._lower_ordered_insts = _patched_lower_ordered_insts


def _split_waits_drain_and_barrier(self, tick_clock, wait_clock):
    nc = self.nc
    probe = nc.sync.nop(nofuse=True)
    wait_clock.add_sem_waits(
        probe.ins, ScopedClock({None: tick_clock.global_clock}))
    si = probe.ins.sync_info
    waits = list(si.on_wait) if si is not None else []
    if len(waits) > _MAX_WAITS:
        si.on_wait = waits[:_MAX_WAITS]
        for i in range(_MAX_WAITS, len(waits), _MAX_WAITS):
            nxt = nc.sync.nop(nofuse=True)
            nxt.ins.sync_info = bass_rust.SyncInfo(
                on_wait=waits[i:i + _MAX_WAITS], on_update=[])
    nc.sync.drain()
    nc.all_engine_barrier()
    assert self.sems is not None
    popped = nc._tile_sem_poison_stack.pop()
    assert popped is self._sem_poison
    nc.clear_and_free_semaphores(list(self.sems.allocated().values()))
    nc.all_engine_barrier()


tile.TileContext._drain_and_barrier = _split_waits_drain_and_barrier

S, D, NCORES = 1536, 1024, 8
P_MAIN = 1024            # compacted+padded tokens per view
KT = D // 128            # 8 contraction tiles
TEMP_INV = 20.0          # 1 / 0.05
FP8_SCALE = 8.0          # f entries ~N(0, 1/32); x8 keeps them in e4m3's
                         # normal range (|f|*8 <~ 2, well under 240)
F32 = mybir.dt.float32
BF16 = mybir.dt.bfloat16
FP8 = mybir.dt.float8e4
AF = mybir.ActivationFunctionType
ALU = mybir.AluOpType
DR = mybir.MatmulPerfMode.DoubleRow


def _build(p: int, num_devices: int = NCORES) -> bass.Bass:
    PT = p // 128        # token tiles per view
    NB2 = 2 * PT         # block rows of the 2P x 2P matrix
    half = p // 512      # column strips per view
    exp_scale = TEMP_INV / (FP8_SCALE * FP8_SCALE)
    pos_scale = TEMP_INV / (FP8_SCALE * FP8_SCALE)

    nc = bass.Bass(num_devices=num_devices)
    h1 = nc.dram_tensor("h1", [p, D], F32, kind="ExternalInput")
    h2 = nc.dram_tensor("h2", [p, D], F32, kind="ExternalInput")
    # mask, pre-laid-out host-side as [128, PT] so token t = 128*col + row
    maskT = nc.dram_tensor("maskT", [128, PT], F32, kind="ExternalInput")
    out = nc.dram_tensor("loss", [1, 1], F32, kind="ExternalOutput")

    with tile.TileContext(nc) as tc, ExitStack() as ctx:
        const_pool = ctx.enter_context(tc.tile_pool(name="const", bufs=1))
        big = ctx.enter_context(tc.tile_pool(name="big", bufs=1))
        stat = ctx.enter_context(tc.tile_pool(name="stat", bufs=1))

        identB = const_pool.tile([128, 128], BF16)
        make_identity(nc, identB[:])
        ones_col = const_pool.tile([128, 1], F32)
        nc.gpsimd.memset(ones_col[:], 1.0)
        ones_sq = const_pool.tile([128, 128], F32)
        nc.gpsimd.memset(ones_sq[:], 1.0)
        ones_bf = const_pool.tile([128, 1], BF16)
        nc.gpsimd.memset(ones_bf[:], 1.0)
        one_f32 = const_pool.tile([1, 1], F32)
        nc.gpsimd.memset(one_f32[:], 1.0)
        msk = const_pool.tile([128, PT], F32)
        nc.sync.dma_start(msk[:], maskT[:])

        # tile-major transposed features: fT[:, t, k*128+c] holds
        # (8*f)^T[d = k*128 + partition, token = t*128 + c] in fp8e4, so
        # each token tile's transpose lands as one contiguous copy
        fT1 = big.tile([128, PT, KT * 128], FP8)
        fT2 = big.tile([128, PT, KT * 128], FP8)
        acc = stat.tile([128, NB2, 2 * half], F32)   # per-strip row sums
        poss20 = stat.tile([128, PT], F32)       # 20 * pos_sim
        csum_sb = stat.tile([1, 2 * p], F32)     # mirror column sums
        msk24 = stat.tile([128, NB2], F32)
        negK0 = stat.tile([128, 1], F32)
        recn = stat.tile([1, 1], F32)

        # zero row-sum slots never written (below-diagonal A/C strips)
        nc.gpsimd.memset(acc[:], 0.0)

        # ---- HAM warmup: keep the PE busy from t~0 so it upclocks.
        # The result is read and spilled to a DRAM scratch so dead-code
        # elimination cannot drop the matmuls.
        wdump = nc.dram_tensor("wdump", [1, 1], F32, kind="Internal")
        with tc.tile_pool(name="warm", bufs=1, space="PSUM") as wp:
            wps = wp.tile([128, 128], F32)
            for i in range(24):
                nc.tensor.matmul(wps[:], identB[:], identB[:],
                                 start=(i == 0), stop=(i == 23))
            wdbg = stat.tile([1, 1], F32)
            nc.vector.tensor_copy(wdbg[:], wps[0:1, 0:1])
            nc.sync.dma_start(wdump[:], wdbg[:])

        # ---- phase 0: mask-only precomputes ----
        with tc.tile_pool(name="ep0", bufs=1) as ep0, \
             tc.tile_pool(name="ep0_ps", bufs=1, space="PSUM") as ep0p:
            msum = ep0.tile([128, 1], F32)
            nc.vector.tensor_reduce(msum[:], msk[:],
                                    axis=mybir.AxisListType.X, op=ALU.add)
            nps = ep0p.tile([128, 1], F32)
            nc.tensor.matmul(nps[:], ones_sq[:], msum[:], start=True,
                             stop=True)
            # -K0 = 2n - 2P, broadcast to all partitions
            nc.scalar.activation(negK0[:], nps[:], AF.Copy, scale=2.0,
                                 bias=float(-2 * p))
            n2c = ep0.tile([1, 1], F32)
            nc.scalar.activation(n2c[:], nps[0:1, :], AF.Copy, scale=2.0)
            nc.vector.reciprocal(recn[:], n2c[:])   # 1/(2n)
            nc.vector.tensor_copy(msk24[:, 0:PT], msk[:])
            nc.vector.tensor_copy(msk24[:, PT:NB2], msk[:])

        with tc.tile_pool(name="mm_ps", bufs=2, space="PSUM") as mmp, \
             tc.tile_pool(name="cng_ps", bufs=1, space="PSUM") as cngp, \
             tc.tile_pool(name="es", bufs=6) as esp, \
             tc.tile_pool(name="ht", bufs=3) as htp, \
             tc.tile_pool(name="scr", bufs=2) as scr, \
             tc.tile_pool(name="sc", bufs=4) as scp, \
             tc.tile_pool(name="tt", bufs=2) as ttp:

            cng = cngp.tile([128, NB2], F32)     # mirror sums, token-major
            pending = []                         # deferred colsum matmuls

            def flush_pending():
                while pending:
                    pending.pop(0)()

            def emit_tile(v, t, src_dram, fT, tps):
                ht = htp.tile([128, D], F32, tag="ht", name=f"ht{v}_{t}")
                nc.sync.dma_start(ht[:], src_dram[t * 128:(t + 1) * 128, :])
                sq = scr.tile([128, D], BF16, tag="sq", name=f"sq{v}_{t}")
                ss = scp.tile([128, 1], F32, tag="ss", name=f"ss{v}_{t}")
                nc.vector.scalar_tensor_tensor(
                    out=sq[:], in0=ht[:], scalar=1.0, in1=ht[:],
                    op0=ALU.mult, op1=ALU.mult, accum_out=ss[:])
                # 1/sqrt(ss) = exp(-0.5*ln(ss)): Ln/Exp share a ScalarE
                # table with the strip exps, so no ACT_TABLE_LOAD thrash
                lnss = scp.tile([128, 1], F32, tag="ln", name=f"ln{v}_{t}")
                nc.scalar.activation(lnss[:], ss[:], AF.Ln)
                sc = scp.tile([128, 1], F32, tag="sc", name=f"sc{v}_{t}")
                nc.scalar.activation(sc[:], lnss[:], AF.Exp, scale=-0.5)
                fnb = scr.tile([128, D], BF16, tag="fn", name=f"fn{v}_{t}")
                nc.vector.tensor_scalar(fnb[:], ht[:], sc[:],
                                        msk[:, t:t + 1],
                                        op0=ALU.mult, op1=ALU.mult)
                pt = tps.tile([128, D], BF16, tag="pt", name=f"pt{v}_{t}")
                for k in range(KT):
                    nc.tensor.transpose(pt[:, k * 128:(k + 1) * 128],
                                        fnb[:, k * 128:(k + 1) * 128],
                                        identB[:])
                # quantize to fp8 (x8) while moving PSUM->SBUF; the
                # tile-major fT makes this a contiguous copy
                nc.scalar.activation(fT[:, t, :], pt[:], AF.Copy,
                                     scale=FP8_SCALE)

            def emit_strip(r, csT, quad, cs_ps, first, last):
                """One sim strip at block-row r (global), local column
                strip csT of quadrant quad. A/C strips containing the
                diagonal are narrowed to skip fully-below-diagonal blocks."""
                lhsT = fT1 if r < PT else fT2
                rT = r % PT
                rhsT = fT1 if quad == "A" else fT2
                ko = 0                            # leading blocks skipped
                if quad != "B" and csT * 512 - rT * 128 < 128:
                    ko = rT - 4 * csT
                nw = 512 - 128 * ko               # strip width
                ps = mmp.tile([128, 512], F32, tag="ps",
                              name=f"ps{quad}_{csT}_{r}")
                rhs4 = rhsT[:, 4 * csT + ko:4 * csT + 4, :].rearrange(
                    "q t (k c) -> q k t c", k=KT)
                lhs3 = lhsT[:, rT, :].rearrange("q (k c) -> q k c", k=KT)
                for g in range(KT // 2):
                    nc.tensor.matmul(
                        ps[:, 0:nw],
                        lhs3[:, 2 * g:2 * g + 2, :],
                        rhs4[:, 2 * g:2 * g + 2, :, :],
                        perf_mode=DR,
                        start=(g == 0), stop=(g == KT // 2 - 1))
                # previous strip's colsum lands here: by now its exp result
                # is ready, so the PE doesn't stall on it
                flush_pending()
                es = esp.tile([128, 512], BF16, tag="es",
                              name=f"es{quad}_{csT}_{r}")
                cs_g = csT if quad == "A" else half + csT
                if quad == "B":
                    nc.scalar.activation(es[:], ps[:], AF.Exp,
                                         scale=exp_scale,
                                         accum_out=acc[:, r, cs_g:cs_g + 1])
                    if csT * 4 <= rT <= csT * 4 + 3:
                        # pos_sim lives on this strip's diagonal block:
                        # extract it exactly from the f32 PSUM sims
                        jb = rT - csT * 4
                        sct = ttp.tile([128, 128], F32, tag="sct",
                                       name=f"sct_{r}")
                        nc.vector.scalar_tensor_tensor(
                            out=sct[:],
                            in0=ps[:, jb * 128:(jb + 1) * 128],
                            scalar=pos_scale,
                            in1=identB[:],
                            op0=ALU.mult,
                            op1=ALU.mult,
                            accum_out=poss20[:, rT:rT + 1])
                elif ko == 0 and csT * 512 - rT * 128 >= 128:
                    # strictly above the diagonal: plain exp + row sums
                    nc.scalar.activation(
                        es[:], ps[:], AF.Exp, scale=exp_scale,
                        accum_out=acc[:, r, cs_g:cs_g + 1])
                else:
                    # first block is the diagonal one: strict upper keep
                    nc.scalar.activation(es[:, 0:nw], ps[:, 0:nw], AF.Exp,
                                         scale=exp_scale)
                    # keep col > row: -1 + (-1)*p + 1*c >= 0
                    nc.gpsimd.affine_select(
                        out=es[:, 0:nw], in_=es[:, 0:nw],
                        compare_op=ALU.is_ge,
                        fill=0.0, base=-1, pattern=[[1, nw]],
                        channel_multiplier=-1)
                    nc.vector.tensor_reduce(acc[:, r, cs_g:cs_g + 1],
                                            es[:, 0:nw],
                                            axis=mybir.AxisListType.X,
                                            op=ALU.add)

                def colsum(es=es, first=first, last=last, vec=cs_ps,
                           ko=ko, nw=nw):
                    nc.tensor.matmul(vec[0:1, ko * 128:ko * 128 + nw],
                                     ones_bf[:], es[:, 0:nw],
                                     start=first, stop=last,
                                     skip_group_check=True)
                pending.append(colsum)

            def emit_mirror(base_chunk, nchunks):
                # transpose csum_sb chunks to token-major via K=1 matmuls
                for c in range(base_chunk, base_chunk + nchunks):
                    nc.tensor.matmul(cng[:, c:c + 1],
                                     csum_sb[0:1, c * 128:(c + 1) * 128],
                                     one_f32[:], start=True, stop=True)

            def a_strips(csT):
                return [(r, "A") for r in range(min(4 * csT + 4, PT))]

            def bc_strips(csT):
                # C diag strips first (their exp->affine->reduce chain
                # overlaps later strips; the emission leader is full
                # width, as the PSUM colsum accumulation group needs),
                # pure-upper C next, B strips last so the drain tail
                # only needs cheap exp+accum
                crows = list(range(min(4 * csT + 4, PT)))
                diag = [(PT + rc, "C") for rc in crows
                        if csT * 512 - rc * 128 < 128]
                pure = [(PT + rc, "C") for rc in crows
                        if csT * 512 - rc * 128 >= 128]
                return diag + pure + [(r, "B") for r in range(PT)]

            # ---- merged schedule: all 16 tiles in sequence, ready
            # strips spread across the remaining tile slots so the PE
            # never starves while tiles DMA/normalize/transpose ----
            with tc.tile_pool(name="tp_ps", bufs=1, space="PSUM") as tps, \
                 tc.tile_pool(name="psA", bufs=1, space="PSUM") as psAp, \
                 tc.tile_pool(name="psB", bufs=1, space="PSUM") as psBp:
                psA = [psAp.tile([1, 512], F32, name=f"psA{c}")
                       for c in range(half)]
                psB = [psBp.tile([1, 512], F32, name=f"psB{c}")
                       for c in range(half)]
                # group bookkeeping: when the last colsum of a column
                # group has been emitted, immediately fold its [1,512]
                # accumulator to token-major mirror columns
                n_a_total = sum(len(a_strips(c)) for c in range(half))
                a_done = 0

                def finish_a():
                    # A colsum groups complete: fold them to token-major
                    flush_pending()
                    for c in range(half):
                        nc.vector.tensor_copy(
                            csum_sb[0:1, c * 512:(c + 1) * 512], psA[c][:])
                    emit_mirror(0, PT)

                def do_strip(s):
                    nonlocal a_done
                    emit_strip(s[0], s[1], s[2], s[3], s[4], s[5])
                    if s[2] == "A":
                        a_done += 1
                        if a_done == n_a_total:
                            finish_a()

                queue = []
                tiles = [(1, t, h1, fT1) for t in range(PT)] + \
                        [(2, t, h2, fT2) for t in range(PT)]
                for i, (v, t, srcd, fT) in enumerate(tiles):
                    emit_tile(v, t, srcd, fT, tps)
                    flush_pending()
                    if t % 4 == 3:
                        csT = t // 4
                        group = a_strips(csT) if v == 1 else bc_strips(csT)
                        vecs = psA if v == 1 else psB
                        for j, (r, qd) in enumerate(group):
                            queue.append((r, csT, qd, vecs[csT],
                                          j == 0, j == len(group) - 1))
                    tiles_left = len(tiles) - 1 - i
                    if tiles_left > 0 and queue:
                        n_emit = -(-len(queue) // (tiles_left + 1))
                        for _ in range(n_emit):
                            do_strip(queue.pop(0))
                for s in queue:
                    do_strip(s)
                flush_pending()
                for c in range(half):
                    nc.vector.tensor_copy(
                        csum_sb[0:1, p + c * 512:p + (c + 1) * 512],
                        psB[c][:])
            emit_mirror(PT, PT)

            # ---- epilogue: final reduction chain ----
            with tc.tile_pool(name="ep", bufs=1) as ep, \
                 tc.tile_pool(name="ep_ps", bufs=1, space="PSUM") as epp:
                cngs = ep.tile([128, NB2], F32)
                nc.vector.tensor_copy(cngs[:], cng[:])
                ng = ep.tile([128, NB2], F32)
                nc.vector.tensor_reduce(ng[:], acc[:],
                                        axis=mybir.AxisListType.X,
                                        op=ALU.add)
                den = ep.tile([128, NB2], F32)
                nc.vector.tensor_add(den[:], ng[:], cngs[:])
                nc.vector.tensor_scalar_add(den[:], den[:], negK0[:])
                lg = ep.tile([128, NB2], F32)
                nc.scalar.activation(lg[:], den[:], AF.Ln)
                pm = ep.tile([128, NB2], F32)
                nc.vector.tensor_copy(pm[:, 0:PT], poss20[:])
                nc.vector.tensor_copy(pm[:, PT:NB2], poss20[:])
                d1 = ep.tile([128, NB2], F32)
                nc.vector.tensor_sub(d1[:], lg[:], pm[:])
                ptok = ep.tile([128, NB2], F32)
                tsum = ep.tile([128, 1], F32)
                nc.vector.scalar_tensor_tensor(
                    out=ptok[:], in0=d1[:], scalar=1.0, in1=msk24[:],
                    op0=ALU.mult, op1=ALU.mult, accum_out=tsum[:])
                lps = epp.tile([1, 1], F32)
                nc.tensor.matmul(lps[:], ones_col[:], tsum[:], start=True,
                                 stop=True)
                lsb = ep.tile([1, 1], F32)
                nc.vector.tensor_mul(lsb[:], lps[:], recn[:])
                nc.sync.dma_start(out[:], lsb[:])

    return nc


_NC = {}


def _get_nc(p: int) -> bass.Bass:
    if p not in _NC:
        _NC[p] = _build(p)
    return _NC[p]


def _mask_layout(mask_col: np.ndarray, p: int) -> np.ndarray:
    # token t = 128 * col + row  ->  [128, PT]
    return np.ascontiguousarray(
        mask_col.astype(np.float32).reshape(p // 128, 128).T)


def _in_maps(h1, h2, mask, p):
    maps = []
    for b in range(NCORES):
        idx = np.argsort(~mask[b], kind="stable")[:p]
        maps.append({
            "h1": np.ascontiguousarray(h1[b][idx]),
            "h2": np.ascontiguousarray(h2[b][idx]),
            "maskT": _mask_layout(mask[b][idx], p),
        })
    return maps


def kernel(last_hidden_states_1, last_hidden_states_2, token_mask_batch):
    h1 = np.ascontiguousarray(np.asarray(last_hidden_states_1,
                                         dtype=np.float32))
    h2 = np.ascontiguousarray(np.asarray(last_hidden_states_2,
                                         dtype=np.float32))
    mask = np.asarray(token_mask_batch).astype(bool)
    assert h1.shape == (NCORES, S, D), h1.shape

    p = P_MAIN if int(mask.sum(axis=1).max()) <= P_MAIN else S
    nc = _get_nc(p)
    res = run_bass_kernel_spmd(nc, _in_maps(h1, h2, mask, p),
                               list(range(NCORES)))
    vals = [np.asarray(res.results[b]["loss"], dtype=np.float32).reshape(())
            for b in range(NCORES)]
    return np.float32(np.mean(vals))
